# revision 10
# baseline (speedup 1.0000x reference)
"""Multi-head self-attention (B=4, N=2048, C=1024, H=16) on 8 Trainium2 NeuronCores.

Sharding v3 (head-split): core c -> (batch b = c//2, head-group g = c%2).
Each core computes Q/K/V for its OWN 8 heads only (no duplicated QKV compute)
over ALL 2048 queries/keys of its batch, runs attention for those heads, and
projects 1024 output rows (queries g*1024..(g+1)*1024).

The projection contracts over all 16 heads' features, so the two cores of a
batch exchange attention outputs: core c sends y(sibling's queries, own heads)
and receives y(own queries, sibling heads). SPMD-uniform exchange: a pairwise
AllReduce(add) over DRAM bounce buffers sums the two cores' (different-query)
chunks elementwise; each core recovers the sibling's data by subtracting its
own contribution. Query order is virtualized host-side (xt columns put the
sibling's queries first), and the wp row order is virtualized per-core so the
received features always land in yt rows 4..7.

Schedule: query-block-outer (vq 0..3), head-pair-inner (hp 0..3); 16 blocks
of 16 k-tiles. QKV chains fill PE slack while ScalarE runs exp (the global
pacer at ~289us); AllReduce #0/#1 fire after vq0/vq1 complete (~30%/55%),
results consumed by the projection at ~80%/tail. PSUM: scores [P,2,512]x2 +
y [65,2,512] + filler [P,512]x2 = 8 banks.
"""
import numpy as np
import ml_dtypes

import concourse.bass as bass
import concourse.mybir as mybir
from concourse import bacc, bass_utils
from concourse.tile import TileContext

B, N, C = 4, 2048, 1024
H, D = 16, 64
P = 128
CT = C // P        # 8 contraction tiles over channels
NQ = N // 2        # 1024 projected rows per core
NK = N             # 2048 keys
KT = NK // P       # 16 key tiles
HPL = 4            # local head pairs (8 own heads)
QB = 512
VQ = N // QB       # 4 query blocks (all 2048 queries)
VA = D + 1         # V augmented with a ones column

BF16 = mybir.dt.bfloat16
F32 = mybir.dt.float32
Exp = mybir.ActivationFunctionType.Exp

_CACHE = {}

# block order: vq-outer, hp-inner
ORDER = [(vq, hp) for vq in range(VQ) for hp in range(HPL)]


def _build():
    nc = bacc.Bacc("TRN2", target_bir_lowering=False, debug=False)

    xt_in = nc.dram_tensor("xt", [VQ, P, CT, QB], BF16, kind="ExternalInput")
    wq_in = nc.dram_tensor("wq", [P, HPL, CT, P], BF16, kind="ExternalInput")
    wk_in = nc.dram_tensor("wk", [P, HPL, CT, P], BF16, kind="ExternalInput")
    wv_in = nc.dram_tensor("wv", [P, CT, QB], BF16, kind="ExternalInput")
    wp_in = nc.dram_tensor("wp", [P, CT, C], BF16, kind="ExternalInput")
    out = nc.dram_tensor("out", [NQ, C], F32, kind="ExternalOutput")

    with TileContext(nc) as tc:
        with (
            tc.tile_pool(name="persist", bufs=1) as pp,
            tc.tile_pool(name="work", bufs=2) as wk_pool,
            tc.tile_pool(name="ps", bufs=1, space="PSUM") as ps,
            tc.tile_pool(name="dram", bufs=1, space="DRAM") as dram,
        ):
            qt = pp.tile([P, HPL, N], BF16)      # Q^T for own 4 head pairs
            ktt = pp.tile([P, HPL, NK], BF16)    # K^T
            vv = pp.tile([P, KT, 8, VA], BF16)   # V (keys on partitions) + ones
            yt = pp.tile([P, CT, NQ], BF16)      # y^T rows: cit 0-3 own, 4-7 recv
            ysend = pp.tile([P, HPL, NQ], F32)   # y^T for sibling's queries
            xt = pp.tile([P, VQ, CT, QB], BF16)
            wq = pp.tile([P, HPL, CT, P], BF16)
            wk = pp.tile([P, HPL, CT, P], BF16)
            wv = pp.tile([P, CT, QB], BF16)
            wp = pp.tile([P, CT, C], BF16)
            cc_in = [dram.tile([P, HPL, QB], F32, name=f"cc_in{i}")
                     for i in range(2)]
            cc_out = [dram.tile([P, HPL, QB], F32, name=f"cc_out{i}")
                      for i in range(2)]

            nc.vector.memset(vv[:, :, :, D:VA], 1.0)

            # input DMA: xt on sync ring; weights on gpsimd ring (scalar
            # queue time is exp time), deadline-ordered
            nc.sync.dma_start(xt[:, 0, 0:4], xt_in[0, :, 0:4])
            nc.scalar.dma_start(xt[:, 0, 4:8], xt_in[0, :, 4:8])
            nc.sync.dma_start(xt[:, 1, 0:4], xt_in[1, :, 0:4])
            nc.scalar.dma_start(xt[:, 1, 4:8], xt_in[1, :, 4:8])
            nc.gpsimd.dma_start(wk[:, 0], wk_in[:, 0])
            nc.gpsimd.dma_start(wq[:, 0], wq_in[:, 0])
            nc.gpsimd.dma_start(wv[:], wv_in[:])
            nc.sync.dma_start(xt[:, 2, 0:4], xt_in[2, :, 0:4])
            nc.scalar.dma_start(xt[:, 2, 4:8], xt_in[2, :, 4:8])
            nc.sync.dma_start(xt[:, 3, 0:4], xt_in[3, :, 0:4])
            nc.scalar.dma_start(xt[:, 3, 4:8], xt_in[3, :, 4:8])
            for hp_i in range(1, HPL):
                nc.gpsimd.dma_start(wk[:, hp_i], wk_in[:, hp_i])
                nc.gpsimd.dma_start(wq[:, hp_i], wq_in[:, hp_i])
            for cit in range(CT):
                nc.gpsimd.dma_start(wp[:, cit], wp_in[:, cit])

            def q_unit(hp, vq):
                def emit():
                    f_ps = ps.tile([P, QB], F32, tag="f", bufs=2, name="f_ps")
                    for ct in range(CT):
                        nc.tensor.matmul(
                            f_ps[:], wq[:, hp, ct, :], xt[:, vq, ct, :],
                            start=(ct == 0), stop=(ct == CT - 1),
                        )
                    nc.vector.tensor_copy(
                        qt[:, hp, vq * QB:(vq + 1) * QB], f_ps[:])
                return emit

            def k_unit(hp, kc):
                def emit():
                    f_ps = ps.tile([P, QB], F32, tag="f", bufs=2, name="f_ps")
                    for ct in range(CT):
                        nc.tensor.matmul(
                            f_ps[:], wk[:, hp, ct, :], xt[:, kc, ct, :],
                            start=(ct == 0), stop=(ct == CT - 1),
                        )
                    nc.vector.tensor_copy(
                        ktt[:, hp, kc * QB:(kc + 1) * QB], f_ps[:])
                return emit

            def v_unit(kt_i):
                def emit():
                    f_ps = ps.tile([P, 8, D], F32, tag="f", bufs=2, name="f_ps")
                    xc, xo = kt_i // 4, (kt_i % 4) * P
                    for ct in range(CT):
                        nc.tensor.matmul(
                            f_ps[:], xt[:, xc, ct, xo:xo + P],
                            wv[:, ct, :],
                            start=(ct == 0), stop=(ct == CT - 1),
                        )
                    nc.vector.tensor_copy(vv[:, kt_i, :, 0:D], f_ps[:])
                return emit

            def proj_unit(nt, coc):
                def emit():
                    o_ps = ps.tile([P, QB], F32, tag="f", bufs=2, name="o_ps")
                    for cit in range(CT):
                        nc.tensor.matmul(
                            o_ps[:],
                            yt[:, cit, nt * P:(nt + 1) * P],
                            wp[:, cit, coc * QB:(coc + 1) * QB],
                            start=(cit == 0), stop=(cit == CT - 1),
                        )
                    o_sb = wk_pool.tile([P, QB], F32, tag="os", bufs=3,
                                        name="o_sb")
                    nc.vector.tensor_copy(o_sb[:], o_ps[:])
                    ring = nc.sync if coc == 0 else nc.scalar
                    ring.dma_start(
                        out[nt * P:(nt + 1) * P, coc * QB:(coc + 1) * QB],
                        o_sb[:],
                    )
                return emit

            # prologue: just enough for the first scores; the rest flows
            # through the (0,0) filler list so exp starts early
            for u in [q_unit(0, 0), k_unit(0, 0), v_unit(0)]:
                u()

            fillers = {
                (0, 0): [v_unit(1), k_unit(0, 1), v_unit(2),
                         v_unit(3), k_unit(0, 2), v_unit(4), v_unit(5),
                         k_unit(0, 3), v_unit(6), v_unit(7), v_unit(8),
                         v_unit(9), v_unit(10), v_unit(11), v_unit(12),
                         v_unit(13), v_unit(14), v_unit(15),
                         q_unit(1, 0), k_unit(1, 0)],
                (0, 1): [k_unit(1, 1), k_unit(1, 2), k_unit(1, 3),
                         q_unit(2, 0), k_unit(2, 0)],
                (0, 2): [k_unit(2, 1), k_unit(2, 2), k_unit(2, 3),
                         q_unit(3, 0), k_unit(3, 0)],
                (0, 3): [k_unit(3, 1), k_unit(3, 2), k_unit(3, 3),
                         q_unit(0, 1), q_unit(1, 1)],
                (1, 0): [q_unit(2, 1), q_unit(3, 1)],
                (1, 1): [q_unit(0, 2), q_unit(1, 2)],
                (1, 2): [q_unit(2, 2), q_unit(3, 2)],
                (1, 3): [q_unit(0, 3), q_unit(1, 3)],
                (2, 0): [q_unit(2, 3), q_unit(3, 3)],
                (2, 1): [],
                (2, 2): [],
                (2, 3): [],
                (3, 0): [proj_unit(0, 0), proj_unit(0, 1)],
                (3, 1): [proj_unit(1, 0), proj_unit(1, 1)],
                (3, 2): [proj_unit(2, 0), proj_unit(2, 1)],
                (3, 3): [proj_unit(3, 0), proj_unit(3, 1)],
            }

            def emit_scores(vq, hp, kt_i):
                ks = slice(kt_i * P, (kt_i + 1) * P)
                qs = slice(vq * QB, (vq + 1) * QB)
                s_ps = ps.tile([P, 2, QB], F32, tag="s", bufs=2, name="s_ps")
                nc.tensor.matmul(
                    s_ps[:, 0], ktt[0:64, hp, ks], qt[0:64, hp, qs],
                    start=True, stop=True, tile_position=(0, 0),
                )
                nc.tensor.matmul(
                    s_ps[:, 1], ktt[64:128, hp, ks], qt[64:128, hp, qs],
                    start=True, stop=True, tile_position=(64, 0),
                )
                return s_ps

            s_cur = emit_scores(0, 0, 0)
            for bi, (vq, hp) in enumerate(ORDER):
                h0, h1 = 2 * hp, 2 * hp + 1
                qs = slice(vq * QB, (vq + 1) * QB)
                if (vq, hp) == (3, 0):
                    # consume AllReduce #0: yt rows 4-7 for own queries 0:512
                    rstage = wk_pool.tile([P, HPL, QB], F32, tag="rs", bufs=2,
                                          name="rstage")
                    nc.sync.dma_start(rstage[:], cc_out[0][:])
                    nc.vector.tensor_sub(yt[:, 4:8, 0:QB], rstage[:],
                                         ysend[:, :, 0:QB])
                pending = list(fillers[(vq, hp)])
                yp = ps.tile([VA, 2, QB], F32, tag="yy", bufs=1, name="yp")
                for kt_i in range(KT):
                    p_sb = wk_pool.tile([P, 2, QB], BF16, tag="pt", bufs=4,
                                        name="p_sb")
                    nc.scalar.activation(p_sb[:], s_cur[:], Exp, scale=0.125)
                    if kt_i < KT - 1:
                        s_cur = emit_scores(vq, hp, kt_i + 1)
                    elif bi + 1 < len(ORDER):
                        nvq, nhp = ORDER[bi + 1]
                        s_cur = emit_scores(nvq, nhp, 0)
                    if 1 <= kt_i <= 13:
                        if pending:
                            pending.pop(0)()
                        if pending and len(pending) > 13 - kt_i:
                            pending.pop(0)()
                    nc.tensor.matmul(
                        yp[:, 0], vv[:, kt_i, h0, :], p_sb[:, 0],
                        start=(kt_i == 0), stop=(kt_i == KT - 1),
                    )
                    nc.tensor.matmul(
                        yp[:, 1], vv[:, kt_i, h1, :], p_sb[:, 1],
                        start=(kt_i == 0), stop=(kt_i == KT - 1),
                    )
                # drain emitted BEFORE leftover fillers: the ycop
                # copies must not queue behind filler CASTs on the DVE FIFO
                # drain: one fast PSUM->SBUF copy frees the y banks, then
                # reciprocal/broadcast/scale SBUF-side. vq 0-1 (sibling's
                # queries) go to ysend (f32); vq 2-3 (own) to yt rows hp.
                ycop = wk_pool.tile([P, QB], F32, tag="yr", bufs=1,
                                    name="ycop")
                dcop = wk_pool.tile([1, 2, QB], F32, tag="dt", bufs=1,
                                    name="dcop")
                rtmp = wk_pool.tile([1, 2, QB], F32, tag="rt", bufs=1,
                                    name="rtmp")
                rtile = wk_pool.tile([P, 2, QB], F32, tag="rr", bufs=1,
                                     name="rtile")
                nc.vector.tensor_copy(ycop[0:64, :], yp[0:D, 0, :])
                nc.vector.tensor_copy(ycop[64:128, :], yp[0:D, 1, :])
                nc.vector.tensor_copy(dcop[:], yp[D:VA, :, :])
                nc.vector.reciprocal_approx_fast(rtmp[:], dcop[:])
                nc.gpsimd.partition_broadcast(rtile[:, 0, :], rtmp[0:1, 0])
                nc.gpsimd.partition_broadcast(rtile[:, 1, :], rtmp[0:1, 1])
                dst = ysend if vq < 2 else yt
                if vq < 2:
                    d0 = dst[0:64, hp, qs]
                    d1 = dst[64:128, hp, qs]
                else:
                    oqs = slice((vq - 2) * QB, (vq - 1) * QB)
                    d0 = dst[0:64, hp, oqs]
                    d1 = dst[64:128, hp, oqs]
                nc.vector.tensor_mul(d0, ycop[0:64, :], rtile[0:64, 0, :])
                nc.vector.tensor_mul(d1, ycop[64:128, :],
                                     rtile[64:128, 1, :])
                while pending:
                    pending.pop(0)()
                if (vq, hp) == (1, 0):
                    # vq0 fully drained (its hp3 muls emitted last block):
                    # ship chunk 0 and fire AllReduce #0
                    nc.gpsimd.dma_start(cc_in[0][:], ysend[:, :, 0:QB])
                    nc.gpsimd.collective_compute(
                        "AllReduce", mybir.AluOpType.add,
                        replica_groups=[[0, 1], [2, 3], [4, 5], [6, 7]],
                        ins=[cc_in[0][:]], outs=[cc_out[0][:]],
                    )
                if (vq, hp) == (2, 0):
                    nc.gpsimd.dma_start(cc_in[1][:], ysend[:, :, QB:NQ])
                    nc.gpsimd.collective_compute(
                        "AllReduce", mybir.AluOpType.add,
                        replica_groups=[[0, 1], [2, 3], [4, 5], [6, 7]],
                        ins=[cc_in[1][:]], outs=[cc_out[1][:]],
                    )

            # consume AllReduce #1, then the projection tail (rows 512:1024)
            rstage = wk_pool.tile([P, HPL, QB], F32, tag="rs", bufs=2,
                                  name="rstage")
            nc.sync.dma_start(rstage[:], cc_out[1][:])
            nc.vector.tensor_sub(yt[:, 4:8, QB:NQ], rstage[:],
                                 ysend[:, :, QB:NQ])
            for nt in range(4, NQ // P):
                for coc in range(2):
                    proj_unit(nt, coc)()
    nc.compile()
    return nc


def _get_nc():
    if "nc" not in _CACHE:
        _CACHE["nc"] = _build()
    return _CACHE["nc"]


def _prep_w(w):
    """[C, F] f32 -> [P, CT', F] bf16 with c = ct*128 + p."""
    c, f = w.shape
    return np.ascontiguousarray(
        w.reshape(c // P, P, f).transpose(1, 0, 2)
    ).astype(ml_dtypes.bfloat16)


def _prep_w_hp(w, g):
    """[C, C] f32 -> own 4 head-pairs [P, 4, CT, P] bf16."""
    full = np.ascontiguousarray(
        w.reshape(CT, P, H // 2, P).transpose(1, 2, 0, 3)
    )
    return np.ascontiguousarray(full[:, g * HPL:(g + 1) * HPL]).astype(
        ml_dtypes.bfloat16)


def _prep_x(xb, g):
    """x[b] [N, C] f32 -> [VQ, P, CT, QB] bf16, sibling's query-half FIRST,
    chunk-contiguous for fast DMA."""
    xT = xb.T  # [C, N]
    o = 1 - g
    perm = np.concatenate(
        [xT[:, o * NQ:(o + 1) * NQ], xT[:, g * NQ:(g + 1) * NQ]], axis=1)
    return np.ascontiguousarray(
        perm.reshape(CT, P, VQ, QB).transpose(2, 1, 0, 3)
    ).astype(ml_dtypes.bfloat16)


def _make_in_maps(x, w_attn, w_proj):
    x = np.asarray(x, dtype=np.float32)
    w_attn = np.asarray(w_attn, dtype=np.float32)
    w_proj = np.asarray(w_proj, dtype=np.float32)
    wv_full = w_attn[:, 2 * C:3 * C]
    in_maps = []
    for c in range(8):
        b, g = c // 2, c % 2
        wp_virt = np.concatenate(
            [w_proj[g * QB:(g + 1) * QB, :],
             w_proj[(1 - g) * QB:(2 - g) * QB, :]], axis=0)
        in_maps.append({
            "xt": _prep_x(x[b], g),
            "wq": _prep_w_hp(w_attn[:, 0:C], g),
            "wk": _prep_w_hp(w_attn[:, C:2 * C], g),
            "wv": _prep_w(wv_full[:, g * QB:(g + 1) * QB]),
            "wp": _prep_w(wp_virt),
        })
    return in_maps


def _run(x, w_attn, w_proj, trace=False):
    nc = _get_nc()
    in_maps = _make_in_maps(x, w_attn, w_proj)
    res = bass_utils.run_bass_kernel_spmd(
        nc, in_maps, core_ids=list(range(8)), trace=trace
    )
    out = np.empty((B, N, C), dtype=np.float32)
    for c in range(8):
        b, half = c // 2, c % 2
        out[b, half * NQ:(half + 1) * NQ, :] = res.results[c]["out"]
    return out, res


def kernel(x, w_attn, w_proj):
    out, _ = _run(x, w_attn, w_proj, trace=False)
    return out


# revision 11
# speedup vs baseline: 1.0453x; 1.0453x over previous
"""Multi-head self-attention (B=4, N=2048, C=1024, H=16) on 8 Trainium2 NeuronCores.

Sharding v3 (head-split): core c -> (batch b = c//2, head-group g = c%2).
Each core computes Q/K/V for its OWN 8 heads only (no duplicated QKV compute)
over ALL 2048 queries/keys of its batch, runs attention for those heads, and
projects 1024 output rows (queries g*1024..(g+1)*1024).

The projection contracts over all 16 heads' features, so the two cores of a
batch exchange attention outputs: core c sends y(sibling's queries, own heads)
and receives y(own queries, sibling heads). SPMD-uniform exchange: a pairwise
AllReduce(add) over DRAM bounce buffers sums the two cores' (different-query)
chunks elementwise; each core recovers the sibling's data by subtracting its
own contribution. Query order is virtualized host-side (xt columns put the
sibling's queries first), and the wp row order is virtualized per-core so the
received features always land in yt rows 4..7.

Schedule: query-block-outer (vq 0..3), head-pair-inner (hp 0..3); 16 blocks
of 16 k-tiles. QKV chains fill PE slack while ScalarE runs exp (the global
pacer at ~289us); AllReduce #0/#1 fire after vq0/vq1 complete (~30%/55%),
results consumed by the projection at ~80%/tail. PSUM: scores [P,2,512]x2 +
y [65,2,512] + filler [P,512]x2 = 8 banks.
"""
import numpy as np
import ml_dtypes

import concourse.bass as bass
import concourse.mybir as mybir
from concourse import bacc, bass_utils
from concourse.tile import TileContext

B, N, C = 4, 2048, 1024
H, D = 16, 64
P = 128
CT = C // P        # 8 contraction tiles over channels
NQ = N // 2        # 1024 projected rows per core
NK = N             # 2048 keys
KT = NK // P       # 16 key tiles
HPL = 4            # local head pairs (8 own heads)
QB = 512
VQ = N // QB       # 4 query blocks (all 2048 queries)
VA = D + 1         # V augmented with a ones column

BF16 = mybir.dt.bfloat16
F32 = mybir.dt.float32
Exp = mybir.ActivationFunctionType.Exp

_CACHE = {}

# block order: vq-outer, hp-inner
ORDER = [(vq, hp) for vq in range(VQ) for hp in range(HPL)]


def _build():
    nc = bacc.Bacc("TRN2", target_bir_lowering=False, debug=False)

    xt_in = nc.dram_tensor("xt", [VQ, P, CT, QB], BF16, kind="ExternalInput")
    wq_in = nc.dram_tensor("wq", [P, HPL, CT, P], BF16, kind="ExternalInput")
    wk_in = nc.dram_tensor("wk", [P, HPL, CT, P], BF16, kind="ExternalInput")
    wv_in = nc.dram_tensor("wv", [P, CT, QB], BF16, kind="ExternalInput")
    wp_in = nc.dram_tensor("wp", [P, CT, C], BF16, kind="ExternalInput")
    out = nc.dram_tensor("out", [NQ, C], F32, kind="ExternalOutput")

    with TileContext(nc) as tc:
        with (
            tc.tile_pool(name="persist", bufs=1) as pp,
            tc.tile_pool(name="work", bufs=2) as wk_pool,
            tc.tile_pool(name="ps", bufs=1, space="PSUM") as ps,
            tc.tile_pool(name="dram", bufs=1, space="DRAM") as dram,
        ):
            qt = pp.tile([P, HPL, N], BF16)      # Q^T for own 4 head pairs
            ktt = pp.tile([P, HPL, NK], BF16)    # K^T
            vv = pp.tile([P, KT, 8, VA], BF16)   # V (keys on partitions) + ones
            yt = pp.tile([P, CT, NQ], BF16)      # y^T rows: cit 0-3 own, 4-7 recv
            ysend = pp.tile([P, HPL, NQ], F32)   # y^T for sibling's queries
            xt = pp.tile([P, VQ, CT, QB], BF16)
            wq = pp.tile([P, HPL, CT, P], BF16)
            wk = pp.tile([P, HPL, CT, P], BF16)
            wv = pp.tile([P, CT, QB], BF16)
            wp = pp.tile([P, CT, C], BF16)
            cc_in = [dram.tile([P, HPL, QB], F32, name=f"cc_in{i}")
                     for i in range(2)]
            cc_out = [dram.tile([P, HPL, QB], F32, name=f"cc_out{i}")
                      for i in range(2)]

            nc.vector.memset(vv[:, :, :, D:VA], 1.0)

            # input DMA: xt on sync ring; weights on gpsimd ring (scalar
            # queue time is exp time), deadline-ordered
            nc.sync.dma_start(xt[:, 0, 0:2], xt_in[0, :, 0:2])
            nc.scalar.dma_start(xt[:, 0, 4:6], xt_in[0, :, 4:6])
            nc.gpsimd.dma_start(wk[:, 0, 0:4], wk_in[:, 0, 0:4])
            nc.sync.dma_start(xt[:, 0, 2:4], xt_in[0, :, 2:4])
            nc.scalar.dma_start(xt[:, 0, 6:8], xt_in[0, :, 6:8])
            nc.gpsimd.dma_start(wk[:, 0, 4:8], wk_in[:, 0, 4:8])
            nc.gpsimd.dma_start(wq[:, 0, 0:4], wq_in[:, 0, 0:4])
            nc.gpsimd.dma_start(wq[:, 0, 4:8], wq_in[:, 0, 4:8])
            nc.sync.dma_start(xt[:, 1, 0:4], xt_in[1, :, 0:4])
            nc.scalar.dma_start(xt[:, 1, 4:8], xt_in[1, :, 4:8])
            nc.gpsimd.dma_start(wv[:], wv_in[:])
            nc.sync.dma_start(xt[:, 2, 0:4], xt_in[2, :, 0:4])
            nc.scalar.dma_start(xt[:, 2, 4:8], xt_in[2, :, 4:8])
            nc.sync.dma_start(xt[:, 3, 0:4], xt_in[3, :, 0:4])
            nc.scalar.dma_start(xt[:, 3, 4:8], xt_in[3, :, 4:8])
            for hp_i in range(1, HPL):
                nc.gpsimd.dma_start(wk[:, hp_i], wk_in[:, hp_i])
                nc.gpsimd.dma_start(wq[:, hp_i], wq_in[:, hp_i])
            for cit in range(CT):
                nc.gpsimd.dma_start(wp[:, cit], wp_in[:, cit])

            def q_unit(hp, vq):
                def emit():
                    f_ps = ps.tile([P, QB], F32, tag="f", bufs=2, name="f_ps")
                    for ct in range(CT):
                        nc.tensor.matmul(
                            f_ps[:], wq[:, hp, ct, :], xt[:, vq, ct, :],
                            start=(ct == 0), stop=(ct == CT - 1),
                        )
                    nc.vector.tensor_copy(
                        qt[:, hp, vq * QB:(vq + 1) * QB], f_ps[:])
                return emit

            def k_unit(hp, kc):
                def emit():
                    f_ps = ps.tile([P, QB], F32, tag="f", bufs=2, name="f_ps")
                    for ct in range(CT):
                        nc.tensor.matmul(
                            f_ps[:], wk[:, hp, ct, :], xt[:, kc, ct, :],
                            start=(ct == 0), stop=(ct == CT - 1),
                        )
                    nc.vector.tensor_copy(
                        ktt[:, hp, kc * QB:(kc + 1) * QB], f_ps[:])
                return emit

            def v_unit(kt_i):
                def emit():
                    f_ps = ps.tile([P, 8, D], F32, tag="f", bufs=2, name="f_ps")
                    xc, xo = kt_i // 4, (kt_i % 4) * P
                    for ct in range(CT):
                        nc.tensor.matmul(
                            f_ps[:], xt[:, xc, ct, xo:xo + P],
                            wv[:, ct, :],
                            start=(ct == 0), stop=(ct == CT - 1),
                        )
                    nc.vector.tensor_copy(vv[:, kt_i, :, 0:D], f_ps[:])
                return emit

            def proj_unit(nt, coc):
                def emit():
                    o_ps = ps.tile([P, QB], F32, tag="f", bufs=2, name="o_ps")
                    for cit in range(CT):
                        nc.tensor.matmul(
                            o_ps[:],
                            yt[:, cit, nt * P:(nt + 1) * P],
                            wp[:, cit, coc * QB:(coc + 1) * QB],
                            start=(cit == 0), stop=(cit == CT - 1),
                        )
                    o_sb = wk_pool.tile([P, QB], F32, tag="os", bufs=3,
                                        name="o_sb")
                    nc.vector.tensor_copy(o_sb[:], o_ps[:])
                    ring = nc.sync if coc == 0 else nc.scalar
                    ring.dma_start(
                        out[nt * P:(nt + 1) * P, coc * QB:(coc + 1) * QB],
                        o_sb[:],
                    )
                return emit

            # prologue: just enough for the first scores; the rest flows
            # through the (0,0) filler list so exp starts early
            for u in [q_unit(0, 0), k_unit(0, 0), v_unit(0)]:
                u()

            fillers = {
                (0, 0): [v_unit(1), k_unit(0, 1), v_unit(2),
                         v_unit(3), k_unit(0, 2), v_unit(4), v_unit(5),
                         k_unit(0, 3), v_unit(6), v_unit(7), v_unit(8),
                         v_unit(9), v_unit(10), v_unit(11), v_unit(12),
                         v_unit(13), v_unit(14), v_unit(15),
                         q_unit(1, 0), k_unit(1, 0)],
                (0, 1): [k_unit(1, 1), k_unit(1, 2), k_unit(1, 3),
                         q_unit(2, 0), k_unit(2, 0)],
                (0, 2): [k_unit(2, 1), k_unit(2, 2), k_unit(2, 3),
                         q_unit(3, 0), k_unit(3, 0)],
                (0, 3): [k_unit(3, 1), k_unit(3, 2), k_unit(3, 3),
                         q_unit(0, 1), q_unit(1, 1)],
                (1, 0): [q_unit(2, 1), q_unit(3, 1)],
                (1, 1): [q_unit(0, 2), q_unit(1, 2)],
                (1, 2): [q_unit(2, 2), q_unit(3, 2)],
                (1, 3): [q_unit(0, 3), q_unit(1, 3)],
                (2, 0): [q_unit(2, 3), q_unit(3, 3)],
                (2, 1): [],
                (2, 2): [],
                (2, 3): [],
                (3, 0): [proj_unit(0, 0), proj_unit(0, 1)],
                (3, 1): [proj_unit(1, 0), proj_unit(1, 1)],
                (3, 2): [proj_unit(2, 0), proj_unit(2, 1)],
                (3, 3): [proj_unit(3, 0), proj_unit(3, 1)],
            }

            def emit_scores(vq, hp, kt_i):
                ks = slice(kt_i * P, (kt_i + 1) * P)
                qs = slice(vq * QB, (vq + 1) * QB)
                s_ps = ps.tile([P, 2, QB], F32, tag="s", bufs=2, name="s_ps")
                nc.tensor.matmul(
                    s_ps[:, 0], ktt[0:64, hp, ks], qt[0:64, hp, qs],
                    start=True, stop=True, tile_position=(0, 0),
                )
                nc.tensor.matmul(
                    s_ps[:, 1], ktt[64:128, hp, ks], qt[64:128, hp, qs],
                    start=True, stop=True, tile_position=(64, 0),
                )
                return s_ps

            s_cur = emit_scores(0, 0, 0)
            for bi, (vq, hp) in enumerate(ORDER):
                h0, h1 = 2 * hp, 2 * hp + 1
                qs = slice(vq * QB, (vq + 1) * QB)
                if (vq, hp) == (3, 0):
                    # consume AllReduce #0: yt rows 4-7 for own queries 0:512
                    rstage = wk_pool.tile([P, HPL, QB], F32, tag="rs", bufs=2,
                                          name="rstage")
                    nc.sync.dma_start(rstage[:], cc_out[0][:])
                    nc.vector.tensor_sub(yt[:, 4:8, 0:QB], rstage[:],
                                         ysend[:, :, 0:QB])
                pending = list(fillers[(vq, hp)])
                y0t = ps.tile([VA, QB], F32, tag="yy0", bufs=1, name="y0t")
                y1t = ps.tile([VA, QB], F32, tag="yy1", bufs=1, name="y1t")
                for kt_i in range(KT):
                    p_sb = wk_pool.tile([P, 2, QB], BF16, tag="pt", bufs=4,
                                        name="p_sb")
                    nc.scalar.activation(p_sb[:], s_cur[:], Exp, scale=0.125)
                    if kt_i < KT - 1:
                        s_cur = emit_scores(vq, hp, kt_i + 1)
                    elif bi + 1 < len(ORDER):
                        nvq, nhp = ORDER[bi + 1]
                        s_cur = emit_scores(nvq, nhp, 0)
                    if 1 <= kt_i <= 13:
                        if pending:
                            pending.pop(0)()
                        if pending and len(pending) > 13 - kt_i:
                            pending.pop(0)()
                    nc.tensor.matmul(
                        y0t[:], vv[:, kt_i, h0, :], p_sb[:, 0],
                        start=(kt_i == 0), stop=(kt_i == KT - 1),
                    )
                    nc.tensor.matmul(
                        y1t[:], vv[:, kt_i, h1, :], p_sb[:, 1],
                        start=(kt_i == 0), stop=(kt_i == KT - 1),
                    )
                # drain emitted BEFORE leftover fillers: the ycop
                # copies must not queue behind filler CASTs on the DVE FIFO
                # drain: one fast PSUM->SBUF copy frees the y banks, then
                # reciprocal/broadcast/scale SBUF-side. vq 0-1 (sibling's
                # queries) go to ysend (f32); vq 2-3 (own) to yt rows hp.
                ycop = wk_pool.tile([P, QB], F32, tag="yr", bufs=1,
                                    name="ycop")
                dcop = wk_pool.tile([1, 2, QB], F32, tag="dt", bufs=1,
                                    name="dcop")
                rtmp = wk_pool.tile([1, 2, QB], F32, tag="rt", bufs=1,
                                    name="rtmp")
                rtile = wk_pool.tile([P, 2, QB], F32, tag="rr", bufs=1,
                                     name="rtile")
                nc.vector.tensor_copy(ycop[0:64, :], y0t[0:D, :])
                nc.vector.tensor_copy(dcop[0:1, 0, :], y0t[D:VA, :])
                nc.vector.tensor_copy(ycop[64:128, :], y1t[0:D, :])
                nc.vector.tensor_copy(dcop[0:1, 1, :], y1t[D:VA, :])
                nc.vector.reciprocal_approx_fast(rtmp[:], dcop[:])
                nc.gpsimd.partition_broadcast(rtile[:, 0, :], rtmp[0:1, 0])
                nc.gpsimd.partition_broadcast(rtile[:, 1, :], rtmp[0:1, 1])
                dst = ysend if vq < 2 else yt
                if vq < 2:
                    d0 = dst[0:64, hp, qs]
                    d1 = dst[64:128, hp, qs]
                else:
                    oqs = slice((vq - 2) * QB, (vq - 1) * QB)
                    d0 = dst[0:64, hp, oqs]
                    d1 = dst[64:128, hp, oqs]
                nc.vector.tensor_mul(d0, ycop[0:64, :], rtile[0:64, 0, :])
                nc.vector.tensor_mul(d1, ycop[64:128, :],
                                     rtile[64:128, 1, :])
                while pending:
                    pending.pop(0)()
                if (vq, hp) == (1, 0):
                    # vq0 fully drained (its hp3 muls emitted last block):
                    # ship chunk 0 and fire AllReduce #0
                    nc.gpsimd.dma_start(cc_in[0][:], ysend[:, :, 0:QB])
                    nc.gpsimd.collective_compute(
                        "AllReduce", mybir.AluOpType.add,
                        replica_groups=[[0, 1], [2, 3], [4, 5], [6, 7]],
                        ins=[cc_in[0][:]], outs=[cc_out[0][:]],
                    )
                if (vq, hp) == (2, 0):
                    nc.gpsimd.dma_start(cc_in[1][:], ysend[:, :, QB:NQ])
                    nc.gpsimd.collective_compute(
                        "AllReduce", mybir.AluOpType.add,
                        replica_groups=[[0, 1], [2, 3], [4, 5], [6, 7]],
                        ins=[cc_in[1][:]], outs=[cc_out[1][:]],
                    )

            # consume AllReduce #1, then the projection tail (rows 512:1024)
            rstage = wk_pool.tile([P, HPL, QB], F32, tag="rs", bufs=2,
                                  name="rstage")
            nc.sync.dma_start(rstage[:], cc_out[1][:])
            nc.vector.tensor_sub(yt[:, 4:8, QB:NQ], rstage[:],
                                 ysend[:, :, QB:NQ])
            for nt in range(4, NQ // P):
                for coc in range(2):
                    proj_unit(nt, coc)()
    nc.compile()
    return nc


def _get_nc():
    if "nc" not in _CACHE:
        _CACHE["nc"] = _build()
    return _CACHE["nc"]


def _prep_w(w):
    """[C, F] f32 -> [P, CT', F] bf16 with c = ct*128 + p."""
    c, f = w.shape
    return np.ascontiguousarray(
        w.reshape(c // P, P, f).transpose(1, 0, 2)
    ).astype(ml_dtypes.bfloat16)


def _prep_w_hp(w, g):
    """[C, C] f32 -> own 4 head-pairs [P, 4, CT, P] bf16."""
    full = np.ascontiguousarray(
        w.reshape(CT, P, H // 2, P).transpose(1, 2, 0, 3)
    )
    return np.ascontiguousarray(full[:, g * HPL:(g + 1) * HPL]).astype(
        ml_dtypes.bfloat16)


def _prep_x(xb, g):
    """x[b] [N, C] f32 -> [VQ, P, CT, QB] bf16, sibling's query-half FIRST,
    chunk-contiguous for fast DMA."""
    xT = xb.T  # [C, N]
    o = 1 - g
    perm = np.concatenate(
        [xT[:, o * NQ:(o + 1) * NQ], xT[:, g * NQ:(g + 1) * NQ]], axis=1)
    return np.ascontiguousarray(
        perm.reshape(CT, P, VQ, QB).transpose(2, 1, 0, 3)
    ).astype(ml_dtypes.bfloat16)


def _make_in_maps(x, w_attn, w_proj):
    x = np.asarray(x, dtype=np.float32)
    w_attn = np.asarray(w_attn, dtype=np.float32)
    w_proj = np.asarray(w_proj, dtype=np.float32)
    wv_full = w_attn[:, 2 * C:3 * C]
    in_maps = []
    for c in range(8):
        b, g = c // 2, c % 2
        wp_virt = np.concatenate(
            [w_proj[g * QB:(g + 1) * QB, :],
             w_proj[(1 - g) * QB:(2 - g) * QB, :]], axis=0)
        in_maps.append({
            "xt": _prep_x(x[b], g),
            "wq": _prep_w_hp(w_attn[:, 0:C], g),
            "wk": _prep_w_hp(w_attn[:, C:2 * C], g),
            "wv": _prep_w(wv_full[:, g * QB:(g + 1) * QB]),
            "wp": _prep_w(wp_virt),
        })
    return in_maps


def _run(x, w_attn, w_proj, trace=False):
    nc = _get_nc()
    in_maps = _make_in_maps(x, w_attn, w_proj)
    res = bass_utils.run_bass_kernel_spmd(
        nc, in_maps, core_ids=list(range(8)), trace=trace
    )
    out = np.empty((B, N, C), dtype=np.float32)
    for c in range(8):
        b, half = c // 2, c % 2
        out[b, half * NQ:(half + 1) * NQ, :] = res.results[c]["out"]
    return out, res


def kernel(x, w_attn, w_proj):
    out, _ = _run(x, w_attn, w_proj, trace=False)
    return out
